# revision 1
# baseline (speedup 1.0000x reference)
"""EarthAttention3D Trainium2 Bass kernel (8 NeuronCores, window-parallel).

930 windows padded to 936 = 8*117; each core runs 117 windows.

Per window (N=144 tokens, C=192, H=6 heads, hd=32), all matmuls bf16:
  qk^T  : PE, W1-qk columns stationary over host-pretransposed x^T (K=C+1,
          ones row carries the qkv bias; q columns pre-scaled by hd^-0.5).
          M-tiles of 64 so every head's q^T/k^T lands at partition base
          0 or 32 -> S matmuls use only row groups 0/1.
  v     : PE, x^T stationary over W1-v columns (v bias folded into proj bias
          via the softmax row-sum identity)
  S^T_h : PE, K=32 row-tiled 2-way (even heads group 0, odd heads group 1).
          PSUM bank layout keeps each row group in its own bank: concurrent
          row-tiled matmuls in the same bank are a fatal PSUM collision, and
          Tile's tracker cannot see PE-PE concurrency.
          16-row token tails of 4 consecutive windows are stacked at
          32-aligned partition offsets of group-persistent PSUM/SBUF tiles so
          tail elementwise ops amortize 4x.
  attn^T = exp(S^T) * exp(mask)^T * exp(bias)^T
          exp on ScalarE (PSUM->SBUF bf16), *em on VectorE (broadcast AP
          across heads), *eb on GpSimd (SBUF only).
  PV    : PE, lhsT=attn^T, rhs=[v_h|1] per head (ones column -> row sums).
          The 16-row K-tail operands are DMA-moved to partition base 0 first
          so tail-K matmuls share row group 0 with body-K and accumulate into
          the same PSUM bank safely; normalization (per-partition reciprocal
          of the sums column, broadcast per head via strided APs) is fused
          into the PSUM eviction on VectorE.
  proj  : attn_out + ones col -> DMA-xbar transpose -> PE with Wp=[proj_w^T;
          pb + proj_w@bv]

DMA instructions carry a ~625ns serialized HWDGE fixed cost, so loads and
stores are batched per 4-window group (single strided DMAs) and the tail
transposes cover all four windows at once.
"""

import sys

import numpy as np

sys.path.insert(0, "/opt/trn_rl_repo")

import ml_dtypes

DIM = 192
H = 6
HD = 32
WINDOW = (2, 6, 12)
N = 144
B_WIN = 930
NCORES = 8
WPC = 117
PADB = NCORES * WPC
KAUG = DIM + 1  # 193
O_QK = 384
BF = ml_dtypes.bfloat16

# W1 qk column order: M-tiles of 64: [q0 q1 | k0 k1 | q2 q3 | k2 k3 | q4 q5 |
# k4 k5]; head h sits at partition base 32*(h%2) of its M-tile.
_W1_OFF = {("q", 0): 0, ("q", 1): 32, ("k", 0): 64, ("k", 1): 96,
           ("q", 2): 128, ("q", 3): 160, ("k", 2): 192, ("k", 3): 224,
           ("q", 4): 256, ("q", 5): 288, ("k", 4): 320, ("k", 5): 352}
# qk/S/st shared psum tile columns (4 banks of 512 f32):
#   banks 0-1 ([0:432] and [512:944]): qk M-tiles, then reused for S bodies
#   banks 2-3 (1024+...): S tails, group-persistent
_MT_COL = [0, 144, 288, 512, 656, 800]  # qk M-tile -> psum col
_QC = [0, 0, 288, 288, 576, 576]  # q_h col in evicted sbQK (64, 864)
_KC = [144, 144, 432, 432, 720, 720]
_SCOL = [0, 512, 144, 656, 288, 800]  # S^T_h psum col (bank = h%2)
_ACOL = [0, 432, 144, 576, 288, 720]  # head col block in compact attn sbuf
_HORD = [0, 2, 4, 1, 3, 5]  # head order of the compact attn blocks


def _pos_index():
    wz, wh, ww = WINDOW
    coords = np.stack(
        np.meshgrid(np.arange(wz), np.arange(wh), np.arange(ww), indexing="ij")
    )
    flat = coords.reshape(3, -1)
    rel = flat[:, :, None] - flat[:, None, :]
    rel = np.transpose(rel, (1, 2, 0)).copy()
    rel[:, :, 2] += ww - 1
    rel[:, :, 1] *= 2 * ww - 1
    rel[:, :, 0] *= (2 * ww - 1) * wh * wh
    return rel.sum(-1)


POS_INDEX = _pos_index()


def _host_inputs(x, mask, qkv_w, qkv_b, proj_w, proj_b, bias_table):
    scale = float(HD) ** -0.5
    qkv_w = np.asarray(qkv_w, np.float32)
    qkv_b = np.asarray(qkv_b, np.float32)
    proj_w = np.asarray(proj_w, np.float32)
    proj_b = np.asarray(proj_b, np.float32)

    wq, wk, wv = qkv_w[0:DIM] * scale, qkv_w[DIM : 2 * DIM], qkv_w[2 * DIM :]
    bq, bk, bv = qkv_b[0:DIM] * scale, qkv_b[DIM : 2 * DIM], qkv_b[2 * DIM :]

    w1 = np.zeros((KAUG, 576), np.float32)
    for h in range(H):
        qo, ko = _W1_OFF[("q", h)], _W1_OFF[("k", h)]
        w1[0:DIM, qo : qo + HD] = wq[HD * h : HD * h + HD].T
        w1[DIM, qo : qo + HD] = bq[HD * h : HD * h + HD]
        w1[0:DIM, ko : ko + HD] = wk[HD * h : HD * h + HD].T
        w1[DIM, ko : ko + HD] = bk[HD * h : HD * h + HD]
    w1[0:DIM, O_QK:576] = wv.T
    w1 = np.ascontiguousarray(w1.astype(BF))

    wp = np.zeros((KAUG, DIM), np.float32)
    wp[0:DIM] = proj_w.T
    wp[DIM] = proj_b + proj_w @ bv
    wp = np.ascontiguousarray(wp.astype(BF))

    meantab = np.asarray(bias_table, np.float32).mean(axis=1)  # (3312, 6)
    bias3 = meantab[POS_INDEX.reshape(-1)].reshape(N, N, H)  # [n, m, h]
    ebt3 = np.exp(bias3.transpose(1, 2, 0))  # [m, h, n]
    ebt = np.empty((N, H * N), np.float32)  # compact-attn head order
    for i, h in enumerate(_HORD):
        ebt[:, i * N : (i + 1) * N] = ebt3[:, h, :]
    ebt = np.ascontiguousarray(ebt.astype(BF))

    xp = np.zeros((PADB, N, DIM), np.float32)
    xp[:B_WIN] = x
    xt = np.ones((PADB, KAUG, N), np.float32)
    xt[:, 0:DIM, :] = xp.transpose(0, 2, 1)
    xt = xt.astype(BF).reshape(NCORES, WPC, KAUG, N)

    mp = np.zeros((PADB, N, N), np.float32)
    mp[:B_WIN] = mask
    emt = np.exp(mp.transpose(0, 2, 1)).astype(BF).reshape(NCORES, WPC, N, N)

    return [
        {
            "xT": np.ascontiguousarray(xt[c]),
            "emT": np.ascontiguousarray(emt[c]),
            "ebT": ebt,
            "w1": w1,
            "wp": wp,
        }
        for c in range(NCORES)
    ]


def _strided(ap2d, start, step, count, inner=None):
    """(P, F) AP -> (P, count[, inner]) with free stride `step` from col start."""
    import concourse.bass as bass

    base = ap2d[:, start : start + 1]
    dims = [base.ap[0], [step, count]]
    if inner is not None:
        dims.append(inner)
    return bass.AP(tensor=base.tensor, offset=base.offset, ap=dims)


def _brep(ap2d, reps):
    """(P, F) AP -> (P, reps, F) with a step-0 broadcast middle dim."""
    import concourse.bass as bass

    return bass.AP(tensor=ap2d.tensor, offset=ap2d.offset,
                   ap=[ap2d.ap[0], [0, reps], *ap2d.ap[1:]])


def _build_kernel(tc, y, xT, emT, ebT, w1, wp):
    from contextlib import ExitStack

    import concourse.mybir as mybir

    nc = tc.nc
    FP32 = mybir.dt.float32
    BF16 = mybir.dt.bfloat16
    EXP = mybir.ActivationFunctionType.Exp

    ctx = ExitStack()
    const = ctx.enter_context(tc.tile_pool(name="const", bufs=1))
    xin = ctx.enter_context(tc.tile_pool(name="xin", bufs=8))
    qksp = ctx.enter_context(tc.tile_pool(name="qksp", bufs=3))
    vsb = ctx.enter_context(tc.tile_pool(name="vsb", bufs=5))
    attns = ctx.enter_context(tc.tile_pool(name="attns", bufs=5))
    aos = ctx.enter_context(tc.tile_pool(name="aos", bufs=5))
    pts = ctx.enter_context(tc.tile_pool(name="pts", bufs=6))
    ysbp = ctx.enter_context(tc.tile_pool(name="ysbp", bufs=6))
    tsum = ctx.enter_context(tc.tile_pool(name="tsum", bufs=6))
    embp = ctx.enter_context(tc.tile_pool(name="embp", bufs=5))
    grp = ctx.enter_context(tc.tile_pool(name="grp", bufs=2))
    psA = ctx.enter_context(tc.tile_pool(name="psA", bufs=3, space="PSUM"))
    psS = ctx.enter_context(tc.tile_pool(name="psS", bufs=1, space="PSUM"))
    psG = ctx.enter_context(tc.tile_pool(name="psG", bufs=1, space="PSUM"))

    # constants
    w1a = const.tile([128, 576], BF16)
    w1b = const.tile([65, 576], BF16)
    wpa = const.tile([128, DIM], BF16)
    wpb = const.tile([65, DIM], BF16)
    ebb = const.tile([128, H * N], BF16)
    ebt4 = const.tile([128, H * N], BF16)
    nc.vector.memset(ebt4[:, :], 1.0)
    nc.sync.dma_start(out=w1a, in_=w1[0:128, :])
    nc.sync.dma_start(out=w1b, in_=w1[128:KAUG, :])
    nc.sync.dma_start(out=wpa, in_=wp[0:128, :])
    nc.sync.dma_start(out=wpb, in_=wp[128:KAUG, :])
    nc.sync.dma_start(out=ebb, in_=ebT[0:128, :])
    for j in range(4):
        nc.sync.dma_start(out=ebt4[32 * j : 32 * j + 16, :], in_=ebT[128:N, :])

    # group-persistent tiles: two alternating hoisted sets (double-buffered
    # across groups); memsets initialize never-matmul-written rows once.
    # gps (1 bank): vt [0:192] | pvt [192:390]; y-tails reuse [192:384] after
    # the pvt reads complete.
    sp = psS.tile([128, 2048], FP32)
    gps = psG.tile([128, 512], FP32)
    nc.vector.memset(sp[:, 1024:2048], 0.0)
    nc.vector.memset(gps[:, :], 1.0)
    gsets = []
    for _s in range(2):
        at_t = grp.tile([128, H * N], BF16, tag=f"at_t{_s}")
        em_t = grp.tile([128, N], BF16, tag=f"em_t{_s}")
        nc.vector.memset(em_t[:, :], 1.0)
        aot = grp.tile([128, 256], BF16, tag=f"aot{_s}")
        nc.vector.memset(aot[:, 192:256], 1.0)
        rect = grp.tile([128, 8], FP32, tag=f"rect{_s}")
        yt_sb = grp.tile([128, DIM], FP32, tag=f"yt_sb{_s}")
        vt_sb = grp.tile([128, 198], BF16, tag=f"vt_sb{_s}")
        gsets.append((at_t, em_t, vt_sb, aot, rect, yt_sb))

    n_groups = (WPC + 3) // 4
    for g in range(n_groups):
        gsize = min(4, WPC - 4 * g)
        at_t, em_t, vt_sb, aot, rect, yt_sb = gsets[g % 2]
        ats, vs, aob = [], [], []

        # one DMA per group for x^T / em bodies (HWDGE fixed cost amortized)
        w0 = 4 * g
        xag = xin.tile([128, 4 * N], BF16, tag="xag")
        xbg = xin.tile([65, 4 * N], BF16, tag="xbg")
        emg = xin.tile([128, 4 * N], BF16, tag="emg")
        nc.sync.dma_start(
            out=xag[:, 0 : gsize * N].rearrange("p (w n) -> p w n", w=gsize),
            in_=xT[w0 : w0 + gsize, 0:128, :].rearrange("w p n -> p w n"))
        nc.sync.dma_start(
            out=xbg[:, 0 : gsize * N].rearrange("p (w n) -> p w n", w=gsize),
            in_=xT[w0 : w0 + gsize, 128:KAUG, :].rearrange("w p n -> p w n"))
        nc.sync.dma_start(
            out=emg[:, 0 : gsize * N].rearrange("p (w n) -> p w n", w=gsize),
            in_=emT[w0 : w0 + gsize, 0:128, :].rearrange("w p n -> p w n"))
        ysbg = ysbp.tile([128, 4 * DIM], FP32, tag="ysbg")

        # ---------------- pass 1: qkv, S^T, exp/mul (bodies) ----------------
        for j in range(gsize):
            w = 4 * g + j
            jo = 32 * j

            xa = xag[:, j * N : (j + 1) * N]
            xb = xbg[:, j * N : (j + 1) * N]
            em = emg[:, j * N : (j + 1) * N]
            nc.sync.dma_start(out=em_t[jo : jo + 16, :], in_=emT[w, 128:N, :])

            # qk^T: 6 M-tiles of 64 into banks 0-1 of sp
            for mt in range(6):
                col = _MT_COL[mt]
                nc.tensor.matmul(sp[0:64, col : col + N],
                                 w1a[:, 64 * mt : 64 * mt + 64], xa,
                                 start=True, stop=False)
                nc.tensor.matmul(sp[0:64, col : col + N],
                                 w1b[:, 64 * mt : 64 * mt + 64], xb,
                                 start=False, stop=True)
            qk_sb = qksp.tile([64, 864], BF16)
            nc.scalar.copy(qk_sb[:, 0:432], sp[0:64, 0:432])
            nc.vector.tensor_copy(qk_sb[:, 432:864], sp[0:64, 512:944])

            # v natural; [v_h | 1] interleave on eviction
            vb = psA.tile([128, DIM], FP32, tag="ps")
            nc.tensor.matmul(vb[:, :], xa[:, 0:128], w1a[:, O_QK:576],
                             start=True, stop=False)
            nc.tensor.matmul(vb[:, :], xb[0:64, 0:128], w1b[0:64, O_QK:576],
                             start=False, stop=True)
            v_sb = vsb.tile([128, 198], BF16)
            nc.vector.memset(v_sb[:, :], 1.0)
            nc.vector.tensor_copy(
                _strided(v_sb[:, :], 0, 33, H, [1, HD]),
                vb[:, :].rearrange("p (h d) -> p h d", h=H),
            )
            nc.tensor.matmul(gps[jo : jo + 16, 0:DIM], xa[:, 128:N],
                             w1a[:, O_QK:576], start=True, stop=False,
                             tile_position=(0, jo))
            nc.tensor.matmul(gps[jo : jo + 16, 0:DIM], xb[0:64, 128:N],
                             w1b[0:64, O_QK:576], start=False, stop=True,
                             tile_position=(0, jo))

            # S^T per head: row group h%2, psum bank h%2 (banks 0-1 of sp,
            # reusing the qk columns after eviction), tails into banks 2-3
            for h in (0, 2, 4, 1, 3, 5):
                base = 32 * (h % 2)
                qT = qk_sb[base : base + 32, _QC[h] : _QC[h] + N]
                kT = qk_sb[base : base + 32, _KC[h] : _KC[h] + N]
                nc.tensor.matmul(sp[:, _SCOL[h] : _SCOL[h] + N],
                                 kT[:, 0:128], qT, start=True, stop=True,
                                 tile_position=(base, 0))
                nc.tensor.matmul(
                    sp[jo : jo + 16, 1024 + _SCOL[h] : 1024 + _SCOL[h] + N],
                    kT[:, 128:N], qT, start=True, stop=True,
                    tile_position=(base, jo))

            # exp -> *em (DVE) -> *eb (GpSimd)
            at = attns.tile([128, H * N], BF16)
            nc.scalar.activation(at[:, 0:432], sp[:, 0:432], EXP)
            nc.vector.tensor_mul(
                at[:, 0:432].rearrange("p (h n) -> p h n", h=3),
                at[:, 0:432].rearrange("p (h n) -> p h n", h=3),
                _brep(em, 3),
            )
            nc.gpsimd.tensor_mul(at[:, 0:432], at[:, 0:432], ebb[:, 0:432])
            nc.scalar.activation(at[:, 432:864], sp[:, 512:944], EXP)
            nc.vector.tensor_mul(
                at[:, 432:864].rearrange("p (h n) -> p h n", h=3),
                at[:, 432:864].rearrange("p (h n) -> p h n", h=3),
                _brep(em, 3),
            )
            nc.gpsimd.tensor_mul(at[:, 432:864], at[:, 432:864],
                                 ebb[:, 432:864])
            ats.append(at)
            vs.append(v_sb)

        # ---------------- group: tails exp/mul, v-tail eviction --------------
        nc.scalar.activation(at_t[:, 0:432], sp[:, 1024:1456], EXP)
        nc.scalar.activation(at_t[:, 432:864], sp[:, 1536:1968], EXP)
        nc.vector.tensor_mul(
            at_t[:, 0:864].rearrange("p (h n) -> p h n", h=H),
            at_t[:, 0:864].rearrange("p (h n) -> p h n", h=H),
            _brep(em_t[:, :], H),
        )
        nc.gpsimd.tensor_mul(at_t[:, 0:864], at_t[:, 0:864], ebt4[:, :])
        nc.vector.memset(vt_sb[:, :], 1.0)
        nc.vector.tensor_copy(
            _strided(vt_sb[:, :], 0, 33, H, [1, HD]),
            gps[:, 0:DIM].rearrange("p (h d) -> p h d", h=H),
        )

        # ---------------- pass 2: PV + normalize (bodies) --------------------
        # prefetch 16-row tail operands to partition base 0 (row group 0:
        # serial with body-K matmuls -> same-bank accumulate is safe)
        at0s, vt0s = [], []
        for j in range(gsize):
            jo = 32 * j
            at0 = tsum.tile([16, H * N], BF16, tag="at0")
            vt0 = tsum.tile([16, 198], BF16, tag="vt0")
            nc.sync.dma_start(out=at0, in_=at_t[jo : jo + 16, :])
            nc.sync.dma_start(out=vt0, in_=vt_sb[jo : jo + 16, :])
            at0s.append(at0)
            vt0s.append(vt0)
        for j in range(gsize):
            jo = 32 * j
            at, v_sb = ats[j], vs[j]
            at0, vt0 = at0s[j], vt0s[j]
            pv_a = psA.tile([128, 198], FP32, tag="ps")
            for h in range(H):
                ac = _ACOL[h]
                nc.tensor.matmul(pv_a[:, 33 * h : 33 * h + 33],
                                 at[:, ac : ac + 128],
                                 v_sb[:, 33 * h : 33 * h + 33],
                                 start=True, stop=False, tile_position=(0, 0))
                nc.tensor.matmul(pv_a[:, 33 * h : 33 * h + 33],
                                 at0[:, ac : ac + 128],
                                 vt0[:, 33 * h : 33 * h + 33],
                                 start=False, stop=True, tile_position=(0, 0))
                # tail M-tile -> pvt (gps bank0), both K-tiles group 0
                nc.tensor.matmul(
                    gps[jo : jo + 16, 192 + 33 * h : 225 + 33 * h],
                    at[:, ac + 128 : ac + N],
                    v_sb[:, 33 * h : 33 * h + 33],
                    start=True, stop=False, tile_position=(0, jo))
                nc.tensor.matmul(
                    gps[jo : jo + 16, 192 + 33 * h : 225 + 33 * h],
                    at0[:, ac + 128 : ac + N],
                    vt0[:, 33 * h : 33 * h + 33],
                    start=False, stop=True, tile_position=(0, jo))
            rec = ysbp.tile([128, 8], FP32, tag="rec")
            nc.vector.reciprocal(rec[:, 0:H], _strided(pv_a[:, :], 32, 33, H))
            ao = aos.tile([128, 256], BF16)
            nc.vector.memset(ao[:, 192:256], 1.0)
            nc.vector.tensor_mul(
                ao[:, 0:192].rearrange("p (h d) -> p h d", h=H),
                _strided(pv_a[:, :], 0, 33, H, [1, HD]),
                _strided(rec[:, :], 0, 1, H, [0, HD]),
            )
            aob.append(ao)

        # group: normalize stacked pv tails -> aot
        nc.vector.reciprocal(rect[:, 0:H],
                             _strided(gps[:, :], 192 + 32, 33, H))
        nc.vector.tensor_mul(
            aot[:, 0:192].rearrange("p (h d) -> p h d", h=H),
            _strided(gps[:, :], 192, 33, H, [1, HD]),
            _strided(rect[:, :], 0, 1, H, [0, HD]),
        )

        # ---------------- pass 3: transpose + proj + output ------------------
        # one 3D transpose covers every window's 16-token tail: the 2-deep
        # middle dim holds the two 128-col c-blocks (out[c,j,n]=in[n,128j+c])
        ptt = pts.tile([128, 256], BF16, tag="ptt")
        nc.sync.dma_start_transpose(
            ptt[:, :].rearrange("p (b n) -> p b n", b=2), aot[:, :])
        for j in range(gsize):
            w = 4 * g + j
            jo = 32 * j
            ao = aob[j]
            ptb = pts.tile([128, 256], BF16)
            nc.sync.dma_start_transpose(
                ptb[:, :].rearrange("p (b n) -> p b n", b=2), ao[:, :])

            yb = psA.tile([128, DIM], FP32, tag="ps")
            nc.tensor.matmul(yb, ptb[:, 0:128], wpa, start=True, stop=False)
            nc.tensor.matmul(yb, ptb[0:65, 128:256], wpb,
                             start=False, stop=True)
            nc.scalar.copy(ysbg[:, j * DIM : (j + 1) * DIM], yb)
            nc.tensor.matmul(gps[jo : jo + 16, 192 : 192 + DIM],
                             ptt[:, jo : jo + 16],
                             wpa, start=True, stop=False, tile_position=(0, jo))
            nc.tensor.matmul(gps[jo : jo + 16, 192 : 192 + DIM],
                             ptt[0:65, 128 + jo : 128 + jo + 16], wpb,
                             start=False, stop=True, tile_position=(0, jo))
        nc.sync.dma_start(
            out=y[w0 : w0 + gsize, 0:128, :].rearrange("w p o -> p w o"),
            in_=ysbg[:, 0 : gsize * DIM].rearrange("p (w o) -> p w o", w=gsize))

        # group: y tails out
        nc.scalar.copy(yt_sb[:, :], gps[:, 192 : 192 + DIM])
        for j in range(gsize):
            w = 4 * g + j
            nc.sync.dma_start(out=y[w, 128:N, :],
                              in_=yt_sb[32 * j : 32 * j + 16, :])

    ctx.close()


_CACHE = {}


def _get_compiled():
    if "nc" in _CACHE:
        return _CACHE["nc"]
    import concourse.tile as tile
    import concourse.mybir as mybir
    from concourse import bacc

    nc = bacc.Bacc("TRN2", target_bir_lowering=False, debug=False,
                   enable_asserts=False, num_devices=NCORES)
    BF16 = mybir.dt.bfloat16
    xT = nc.dram_tensor("xT", (WPC, KAUG, N), BF16, kind="ExternalInput").ap()
    emT = nc.dram_tensor("emT", (WPC, N, N), BF16, kind="ExternalInput").ap()
    ebT = nc.dram_tensor("ebT", (N, H * N), BF16, kind="ExternalInput").ap()
    w1 = nc.dram_tensor("w1", (KAUG, 576), BF16, kind="ExternalInput").ap()
    wp = nc.dram_tensor("wp", (KAUG, DIM), BF16, kind="ExternalInput").ap()
    y = nc.dram_tensor("y", (WPC, N, DIM), mybir.dt.float32,
                       kind="ExternalOutput").ap()
    with tile.TileContext(nc) as tc:
        _build_kernel(tc, y, xT, emT, ebT, w1, wp)
    nc.compile()
    _CACHE["nc"] = nc
    return nc


def kernel(x, mask, qkv_w, qkv_b, proj_w, proj_b, bias_table):
    from concourse.bass_utils import run_bass_kernel_spmd

    in_maps = _host_inputs(np.asarray(x), np.asarray(mask), qkv_w, qkv_b,
                           proj_w, proj_b, bias_table)
    nc = _get_compiled()
    res = run_bass_kernel_spmd(nc, in_maps, core_ids=list(range(NCORES)))
    out = np.concatenate([r["y"] for r in res.results], axis=0)
    return np.ascontiguousarray(out[:B_WIN]).astype(np.float32)

